# revision 1
# baseline (speedup 1.0000x reference)
"""LinearGCN (y = segment_sum(h[col]*val, row) @ W.T) on 8 Trainium2 NeuronCores.

Strategy: 1D node partition — core m owns output rows [m*12500, (m+1)*12500).
h is replicated (fp16) in every core's HBM, so each core fetches the source
rows for its own edges locally with bulk SWDGE dma_gather across 4 parallel
SWDGE queues (no collectives). Edges are host-bucketed per (256-row
destination block, 25k source-col chunk) and padded to multiples of 128.
Segment-sum runs on the tensor engine as psum_yT += H_tile^T @ S_tile, where
S (one-hot(row)*val, fp16) is host-precomputed and streamed by sequential
HWDGE DMA. A second matmul applies W^T per 128-row half-block.
"""
import sys
import os

sys.path.insert(0, '/opt/trn_rl_repo')

import numpy as np

N_NODES = 100000
N_EDGES = 1600000
D = 128
NC_CORES = 8
NLOC = N_NODES // NC_CORES        # 12500 rows per core
R = 128                            # destination-row block width
NBLK = (NLOC + R - 1) // R         # 98 blocks (97 full + 84 rows)
NCHUNK = 4
CHUNK = N_NODES // NCHUNK          # 25000 source rows per chunk (int16 safe)
GRP = int(os.environ.get('GCN_GRP', '8'))  # blocks per gather group
NGRP = (NBLK + GRP - 1) // GRP     # 13 groups
NQ = 4                             # parallel SWDGE queues


def _preprocess(h, edge_row, edge_col, edge_val, weight):
    """Bucket/pad edges into the common (all-core) stream layout.

    Stream order: for g in groups: for ch in chunks: for b in g: run(b, ch).
    """
    h = np.asarray(h, np.float32)
    edge_row = np.asarray(edge_row, np.int32)
    edge_col = np.asarray(edge_col, np.int32)
    edge_val = np.asarray(edge_val, np.float32)
    weight = np.asarray(weight, np.float32)

    core = edge_row // NLOC
    rloc = edge_row - core * NLOC
    blk = rloc // R
    ch = edge_col // CHUNK
    bucket = (core * NBLK + blk) * NCHUNK + ch
    order = np.lexsort((edge_col, bucket))
    counts = np.bincount(bucket[order], minlength=NC_CORES * NBLK * NCHUNK)
    counts = counts.reshape(NC_CORES, NBLK, NCHUNK)

    # common padded run lengths + stream offsets in (g, ch, b) order
    L = np.max(counts, axis=0)
    L = ((L + 127) // 128) * 128
    off = np.zeros((NBLK, NCHUNK), np.int64)
    call_off = np.zeros((NGRP, NCHUNK), np.int64)
    call_len = np.zeros((NGRP, NCHUNK), np.int64)
    pos = 0
    for g in range(NGRP):
        blks = range(g * GRP, min((g + 1) * GRP, NBLK))
        for c in range(NCHUNK):
            call_off[g, c] = pos
            for b in blks:
                off[b, c] = pos
                pos += L[b, c]
            call_len[g, c] = pos - call_off[g, c]
    e_pad = int(pos)

    # destination slot of every (sorted) edge
    run_start_flat = off.reshape(-1)
    csum = np.concatenate(([0], np.cumsum(counts.reshape(-1))))
    rank = np.arange(len(order)) - np.repeat(csum[:-1], counts.reshape(-1))
    dest = np.repeat(np.tile(run_start_flat, NC_CORES), counts.reshape(-1)) + rank

    col_s = edge_col[order]
    row_s = rloc[order]
    val_s = edge_val[order]
    core_s = core[order]
    blk_s = blk[order]
    ch_s = ch[order]

    gidx = np.zeros((NC_CORES, e_pad), np.int16)
    gidx[core_s, dest] = (col_s - ch_s * CHUNK).astype(np.int16)
    s16 = e_pad // 16
    gidx_w = np.ascontiguousarray(
        np.broadcast_to(
            gidx.reshape(NC_CORES, s16, 16).transpose(0, 2, 1)[:, None, :, :],
            (NC_CORES, 8, 16, s16),
        ).reshape(NC_CORES, 128, s16)
    )
    del gidx

    # host-built one-hot selector stream (fp8e4m3 bit pattern 0x38 == 1.0);
    # edge weights go in a separate per-edge val stream applied to H on-chip
    nt_all = e_pad // 128
    s_full = np.zeros((NC_CORES, e_pad, R), np.uint8)
    s_full[core_s, dest, (row_s - blk_s * R)] = 0x38
    val = np.zeros((NC_CORES, e_pad), np.float16)
    val[core_s, dest] = val_s.astype(np.float16)
    val_w = np.ascontiguousarray(
        val.reshape(NC_CORES, nt_all, 128).transpose(0, 2, 1))
    del val
    # reorder tiles to block-major consumption order: for b: for c: run tiles
    perm = []
    sb_off = np.zeros(NBLK + 1, np.int64)
    for b in range(NBLK):
        sb_off[b] = len(perm)
        for c in range(NCHUNK):
            t0 = int(off[b, c]) // 128
            perm.extend(range(t0, t0 + int(L[b, c]) // 128))
    sb_off[NBLK] = len(perm)
    perm = np.asarray(perm)
    # wrap to [core, 128, nt_all*R]: partition p holds tile-major 256-elem rows
    s_w = np.ascontiguousarray(
        s_full.reshape(NC_CORES, nt_all, 128, R)[:, perm].transpose(0, 2, 1, 3)
    ).reshape(NC_CORES, 128, nt_all * R)
    del s_full

    h16 = h.astype(np.float16)
    wT = np.ascontiguousarray(weight.T.astype(np.float32))

    meta = dict(L=L, off=off, call_off=call_off, call_len=call_len, e_pad=e_pad, sb_off=sb_off)
    ins = dict(h16=h16, gidx=gidx_w, s=s_w, val=val_w, wT=wT)
    return meta, ins


def _build_program(meta):
    from concourse import bacc, tile
    import concourse.mybir as mybir

    L = meta['L']; off = meta['off']
    call_off = meta['call_off']; call_len = meta['call_len']
    e_pad = meta['e_pad']
    nt_all = e_pad // 128

    nc = bacc.Bacc("TRN2", target_bir_lowering=False, debug=False,
                   num_devices=NC_CORES, num_swdge_queues=NQ,
                   dynamic_dma_scratch_size=int(os.environ.get("GCN_SCRATCH", "16384")))
    f16, f32, i16 = mybir.dt.float16, mybir.dt.float32, mybir.dt.int16
    h_d = nc.dram_tensor("h16", [N_NODES, D], f16, kind="ExternalInput")
    gidx_d = nc.dram_tensor("gidx", [128, e_pad // 16], i16, kind="ExternalInput")
    f8 = mybir.dt.float8e4
    s_d = nc.dram_tensor("s", [128, nt_all * R], f8, kind="ExternalInput")
    val_d = nc.dram_tensor("val", [128, nt_all], f16, kind="ExternalInput")
    wT_d = nc.dram_tensor("wT", [D, D], f32, kind="ExternalInput")
    out_d = nc.dram_tensor("out", [NLOC, D], f32, kind="ExternalOutput")

    max_cl = {c: max(int(call_len[g, c]) for g in range(NGRP)) for c in range(NCHUNK)}
    sb_off = meta['sb_off']
    max_bnt = max(int(sb_off[b + 1] - sb_off[b]) for b in range(NBLK))

    qn = 0
    with tile.TileContext(nc) as tc:
        with tc.tile_pool(name="const", bufs=1) as cpool, \
             tc.tile_pool(name="hb", bufs=3) as hpool, \
             tc.tile_pool(name="sst", bufs=3) as sspool, \
             tc.tile_pool(name="y", bufs=2) as ypool, \
             tc.tile_pool(name="o", bufs=3) as opool, \
             tc.tile_pool(name="p1", bufs=6, space="PSUM") as p1pool, \
             tc.tile_pool(name="p2", bufs=2, space="PSUM") as p2pool:
            gidx_t = cpool.tile([128, e_pad // 16], i16)
            nc.sync.dma_start(out=gidx_t[:], in_=gidx_d[:])
            wT_t = cpool.tile([D, D], f32)
            nc.sync.dma_start(out=wT_t[:], in_=wT_d[:])
            val_t = cpool.tile([128, nt_all], f16)
            nc.sync.dma_start(out=val_t[:], in_=val_d[:])

            for g in range(NGRP):
                blks = list(range(g * GRP, min((g + 1) * GRP, NBLK)))
                hbufs = {}
                for c in range(NCHUNK):
                    cl = int(call_len[g, c])
                    if cl == 0:
                        continue
                    hb = hpool.tile([128, max_cl[c] // 128, D], f16, tag=f"hb{c}")
                    co = int(call_off[g, c])
                    nsplit = int(os.environ.get("GCN_SPLIT", "1"))
                    nt_c = cl // 128
                    bounds = [128 * ((nt_c * i) // nsplit) for i in range(nsplit + 1)]
                    for i in range(nsplit):
                        c0, c1 = bounds[i], bounds[i + 1]
                        if c1 == c0:
                            continue
                        nc.gpsimd.dma_gather(
                            hb[:, c0 // 128:c1 // 128, :],
                            h_d[c * CHUNK:(c + 1) * CHUNK, :],
                            gidx_t[:, (co + c0) // 16:(co + c1) // 16],
                            c1 - c0, c1 - c0, D, single_packet=False,
                            queue_num=qn % NQ,
                        )
                        qn += 1
                    nt_call = cl // 128
                    ct0 = co // 128
                    vb = val_t[:, ct0:ct0 + nt_call].unsqueeze(2).broadcast_to(
                        (128, nt_call, D))
                    nc.vector.tensor_tensor(
                        hb[:, :nt_call, :], hb[:, :nt_call, :], vb,
                        mybir.AluOpType.mult)
                    hbufs[c] = hb
                for b in blks:
                    ntiles = int(sum(L[b, c] for c in range(NCHUNK))) // 128
                    rows = min(R, NLOC - b * R)
                    bt0 = int(sb_off[b])
                    s_sb = sspool.tile([128, max_bnt * R], f8, tag="s")
                    if ntiles:
                        nc.sync.dma_start(
                            out=s_sb[:, :ntiles * R],
                            in_=s_d[:, bt0 * R:(bt0 + ntiles) * R])
                    psum1 = p1pool.tile([128, R], f32)
                    k = 0
                    for c in range(NCHUNK):
                        nt = int(L[b, c]) // 128
                        if nt == 0:
                            continue
                        loc_t = (int(off[b, c]) - int(call_off[g, c])) // 128
                        hb = hbufs[c]
                        for t in range(nt):
                            nc.tensor.matmul(
                                psum1[:],
                                lhsT=hb[:, loc_t + t, :],
                                rhs=s_sb[:, k * R:(k + 1) * R],
                                start=(k == 0), stop=(k == ntiles - 1),
                            )
                            k += 1
                    yT_t = ypool.tile([128, R], f32)
                    if ntiles == 0:
                        nc.vector.memset(yT_t[:], 0.0)
                    else:
                        nc.scalar.copy(yT_t[:], psum1[:])
                    m = rows
                    psum2 = p2pool.tile([128, D], f32)
                    nc.tensor.matmul(
                        psum2[:m, :], lhsT=yT_t[:, :m],
                        rhs=wT_t[:], start=True, stop=True,
                    )
                    o_t = opool.tile([128, D], f32)
                    nc.vector.tensor_copy(o_t[:m, :], psum2[:m, :])
                    r0 = b * R
                    nc.sync.dma_start(out=out_d[r0:r0 + m, :], in_=o_t[:m, :])
    nc.compile()
    return nc


def kernel(h, edge_row, edge_col, edge_val, weight):
    meta, ins = _preprocess(h, edge_row, edge_col, edge_val, weight)
    nc = _build_program(meta)

    from concourse.bass_utils import run_bass_kernel_spmd

    in_maps = [
        {"h16": ins["h16"], "gidx": ins["gidx"][m], "s": ins["s"][m],
         "val": ins["val"][m], "wT": ins["wT"]}
        for m in range(NC_CORES)
    ]

    trace = bool(os.environ.get("BASS_GCN_TRACE"))
    if trace:
        import types
        sys.path.insert(0, '/root/.axon_site/trn_agent_boot')
        try:
            from trn_boot import _ntff_profile_via_ctypes
            mod = types.ModuleType('antenv.axon_hooks')
            hook = _ntff_profile_via_ctypes('/opt/axon/libaxon_pjrt.so')
            mod.get_axon_ntff_profile_hook = lambda: hook
            sys.modules['antenv.axon_hooks'] = mod
        except Exception:
            trace = False

    res = run_bass_kernel_spmd(nc, in_maps, list(range(NC_CORES)), trace=trace)
    if trace:
        kernel.last_exec_time_ns = res.exec_time_ns
        kernel.last_results = res
    out = np.concatenate([res.results[m]["out"] for m in range(NC_CORES)], axis=0)
    return out

